# revision 8
# baseline (speedup 1.0000x reference)
"""Trainium2 Bass kernel for a pre-norm transformer block (B=2, S=2048, D=1024, H=16).

Parallelization (8 NeuronCores, SPMD single NEFF):
  - Attention: head-parallel. Core c computes heads {2c, 2c+1} for BOTH batch
    elements (the token axis is flattened to 4096 = [batch0 | batch1]).
  - FFN / LN2 / residual: token-parallel. Core c owns flat token rows
    [512c, 512c+512) (i.e. batch c//4, sequence chunk c%4).
  - One 8-way AllToAll mid-kernel moves per-head attention outputs to the
    token-owner cores. All other data dependencies are core-local.

Layout strategy: activations are kept feature-major ("transposed", [D, S])
for every matmul (the contraction dim must live on SBUF partitions), and
token-major for LayerNorm residual arithmetic. All transposes are done by
the DMA xbar engine (bf16), never by compute engines.

Numerics: matmuls run in bf16 with fp32 PSUM accumulation; softmax,
LayerNorm statistics and the residual stream are fp32.  LN scale/bias are
folded into the weight matrices on the host.
"""

import os
import numpy as np
import ml_dtypes

BF16 = ml_dtypes.bfloat16

B, S, D, H, DH = 2, 2048, 1024, 16, 64
SEQ = B * S                    # 4096 flattened tokens
NCORES = 8
EPS = 1e-5
SCALE = 1.0 / np.sqrt(DH)      # 0.125
ND = D // 128                  # 8 d-tiles
NSC = SEQ // 512               # 8 s-chunks of 512
NTT = SEQ // 128               # 32 t-tiles of 128
CHUNK = SEQ // NCORES          # 512 tokens per core for FFN/residual


def _build_program(has_pm: bool, has_vb: bool):
    import concourse.bass as bass
    import concourse.tile as tile
    from concourse import bacc, mybir

    f32 = mybir.dt.float32
    bf16 = mybir.dt.bfloat16
    AF = mybir.ActivationFunctionType
    ALU = mybir.AluOpType

    nc = bacc.Bacc(
        "TRN2",
        target_bir_lowering=False,
        debug=False,
        enable_asserts=True,
        num_devices=NCORES,
    )

    # ---------------- external I/O ----------------
    xb = nc.dram_tensor("xb", [SEQ, D], bf16, kind="ExternalInput")
    xres_d = nc.dram_tensor("xres", [CHUNK, D], f32, kind="ExternalInput")
    wq_d = nc.dram_tensor("wq", [D, 2 * DH], bf16, kind="ExternalInput")
    wk_d = nc.dram_tensor("wk", [D, 2 * DH], bf16, kind="ExternalInput")
    wv_d = nc.dram_tensor("wv", [D, 2 * DH], bf16, kind="ExternalInput")
    bq_d = nc.dram_tensor("bq", [2 * DH], f32, kind="ExternalInput")
    bk_d = nc.dram_tensor("bk", [2 * DH], f32, kind="ExternalInput")
    w1_d = nc.dram_tensor("w1", [D, D], bf16, kind="ExternalInput")
    b1_d = nc.dram_tensor("b1e", [D], f32, kind="ExternalInput")
    w2_d = nc.dram_tensor("w2", [D, D], bf16, kind="ExternalInput")
    b2_d = nc.dram_tensor("b2e", [D], f32, kind="ExternalInput")
    tri_d = nc.dram_tensor("trimask", [128, 128], bf16, kind="ExternalInput")
    if has_pm:
        pm_d = nc.dram_tensor("pmf", [SEQ], f32, kind="ExternalInput")
    if has_vb:
        vb_d = nc.dram_tensor("vb", [2 * DH], f32, kind="ExternalInput")
    out_d = nc.dram_tensor("out", [CHUNK, D], f32, kind="ExternalOutput")

    def bcast(ap_row, parts):
        """partition-broadcast a [1, N] DRAM row to [parts, N] (DMA-side)."""
        return bass.AP(
            tensor=ap_row.tensor,
            offset=ap_row.offset,
            ap=[[0, parts], ap_row.ap[-1]],
        )

    from contextlib import ExitStack

    with tile.TileContext(nc) as tc, ExitStack() as outer:
        dram = outer.enter_context(tc.tile_pool(name="dram", bufs=1, space="DRAM"))
        consts = outer.enter_context(tc.tile_pool(name="consts", bufs=1))
        sqp = outer.enter_context(tc.tile_pool(name="sqp", bufs=3))
        xrp = outer.enter_context(tc.tile_pool(name="xrp", bufs=4))

        mmps = outer.enter_context(tc.tile_pool(name="mmps", bufs=2, space="PSUM"))
        scps = outer.enter_context(tc.tile_pool(name="scps", bufs=2, space="PSUM"))
        zps = outer.enter_context(tc.tile_pool(name="zps", bufs=2, space="PSUM"))
        auxps = outer.enter_context(tc.tile_pool(name="auxps", bufs=2, space="PSUM"))

        mid = outer.enter_context(ExitStack())
        qkp = mid.enter_context(tc.tile_pool(name="qkp", bufs=1))
        vap = mid.enter_context(tc.tile_pool(name="vap", bufs=NTT))
        pp = mid.enter_context(tc.tile_pool(name="pp", bufs=6))
        ztp = mid.enter_context(tc.tile_pool(name="ztp", bufs=1))
        bcr = mid.enter_context(tc.tile_pool(name="bcr", bufs=2))
        ripr = mid.enter_context(tc.tile_pool(name="ripr", bufs=2))

        ph1 = mid.enter_context(ExitStack())
        xtp = ph1.enter_context(tc.tile_pool(name="xtp", bufs=8))
        rows = ph1.enter_context(tc.tile_pool(name="rows", bufs=2))
        bcp = ph1.enter_context(tc.tile_pool(name="bcp", bufs=1))

        # ------------- constants / weights into SBUF -------------
        wq_sb = consts.tile([128, ND, 2 * DH], bf16)
        wk_sb = consts.tile([128, ND, 2 * DH], bf16)
        wv_sb = consts.tile([128, ND, 2 * DH], bf16)
        for w_sb, w_d in ((wq_sb, wq_d), (wk_sb, wk_d), (wv_sb, wv_d)):
            nc.sync.dma_start(
                out=w_sb,
                in_=w_d.ap().rearrange("(j p) e -> p j e", p=128),
            )
        bq_sb = consts.tile([128, 1], f32)
        bk_sb = consts.tile([128, 1], f32)
        nc.sync.dma_start(out=bq_sb, in_=bq_d.ap().rearrange("(one p) -> p one", one=1))
        nc.sync.dma_start(out=bk_sb, in_=bk_d.ap().rearrange("(one p) -> p one", one=1))
        b1_sb = consts.tile([128, ND], f32)
        b2_sb = consts.tile([128, ND], f32)
        nc.sync.dma_start(out=b1_sb, in_=b1_d.ap().rearrange("(m p) -> p m", p=128))
        nc.sync.dma_start(out=b2_sb, in_=b2_d.ap().rearrange("(m p) -> p m", p=128))
        tri_sb = consts.tile([128, 128], bf16)
        nc.sync.dma_start(out=tri_sb, in_=tri_d.ap())
        ones1_sb = consts.tile([128, 1], bf16)
        nc.vector.memset(ones1_sb, 1.0)
        ones64_sb = consts.tile([1, DH], f32)
        nc.vector.memset(ones64_sb, 1.0)
        eps_sb = consts.tile([1, 1], f32)
        nc.vector.memset(eps_sb, EPS)
        if has_pm:
            pm_sb = consts.tile([128, NTT], f32)
            nc.sync.dma_start(out=pm_sb, in_=pm_d.ap().rearrange("(t p) -> p t", p=128))
        if has_vb:
            vb_sb = consts.tile([128, 2 * DH], f32)
            nc.gpsimd.dma_start(
                out=vb_sb, in_=bcast(vb_d.ap().rearrange("(one e) -> one e", one=1), 128)
            )

        # residual chunk (loaded early; used after the AllToAll)
        xres = []
        for i in range(4):
            t = xrp.tile([128, D], f32, tag="xres")
            nc.sync.dma_start(out=t, in_=xres_d.ap()[128 * i : 128 * (i + 1), :])
            xres.append(t)

        # DRAM scratch
        a2a_in = dram.tile([NCORES * 128, 512], bf16)
        a2a_out = dram.tile([NCORES * 128, 512], bf16)
        xz_dram = dram.tile([CHUNK, D], bf16)
        fft_dram = dram.tile([D, CHUNK], bf16)
        ln1rows_dram = dram.tile([1, 2 * SEQ], bf16)
        ln2rows_dram = dram.tile([1, 2 * CHUNK], bf16)

        # ------------- load x^T (feature-major, bf16) -------------
        xt = []
        for j in range(ND):
            t = xtp.tile([128, SEQ], bf16, tag="xt")
            nc.sync.dma_start_transpose(out=t, in_=xb.ap()[:, 128 * j : 128 * (j + 1)])
            xt.append(t)

        # ------------- LN1 statistics (PE ones-matmuls) -------------
        for sc in range(NSC):
            cs = slice(512 * sc, 512 * (sc + 1))
            sum_ps = auxps.tile([1, 512], f32, tag="aux")
            sq_ps = auxps.tile([1, 512], f32, tag="aux")
            for j in range(ND):
                sq_t = sqp.tile([128, 512], bf16, tag="sq")
                nc.vector.tensor_mul(out=sq_t, in0=xt[j][:, cs], in1=xt[j][:, cs])
                nc.tensor.matmul(
                    out=sum_ps, lhsT=ones1_sb, rhs=xt[j][:, cs],
                    start=(j == 0), stop=(j == ND - 1),
                )
                nc.tensor.matmul(
                    out=sq_ps, lhsT=ones1_sb, rhs=sq_t,
                    start=(j == 0), stop=(j == ND - 1),
                )
            # rows: mu, var, rstd (rstd = exp(-0.5*ln(var+eps)) -> one ACT table set)
            srow = rows.tile([1, 3 * 512], f32, tag="srow")
            lnbf = rows.tile([1, 2 * 512], bf16, tag="lnbf")
            mu_r, var_r, tmp_r = srow[:, 0:512], srow[:, 512:1024], srow[:, 1024:1536]
            nc.scalar.mul(out=mu_r, in_=sum_ps, mul=1.0 / D)
            nc.scalar.mul(out=var_r, in_=sq_ps, mul=1.0 / D)
            nc.vector.tensor_mul(out=tmp_r, in0=mu_r, in1=mu_r)
            nc.vector.tensor_sub(out=var_r, in0=var_r, in1=tmp_r)
            nc.scalar.activation(out=tmp_r, in_=var_r, func=AF.Ln, bias=eps_sb)
            nc.scalar.activation(out=lnbf[:, 512:1024], in_=tmp_r, func=AF.Exp, scale=-0.5)
            nc.vector.tensor_copy(out=lnbf[:, 0:512], in_=mu_r)
            nc.sync.dma_start(
                out=bass.AP(
                    tensor=ln1rows_dram.tensor,
                    offset=ln1rows_dram.offset + 512 * sc,
                    ap=[[1, 1], [SEQ, 2], [1, 512]],
                ),
                in_=lnbf.rearrange("one (two n) -> one two n", two=2),
            )
        mu_b = bcp.tile([128, SEQ], bf16, tag="mu_b")
        rstd_b = bcp.tile([128, SEQ], bf16, tag="rstd_b")
        nc.gpsimd.dma_start(out=mu_b, in_=bcast(ln1rows_dram[:, 0:SEQ], 128))
        nc.gpsimd.dma_start(out=rstd_b, in_=bcast(ln1rows_dram[:, SEQ : 2 * SEQ], 128))
        # apply LN1 in place: xt becomes h^T (LN scale/bias folded into weights)
        for j in range(ND):
            nc.vector.tensor_sub(out=xt[j], in0=xt[j], in1=mu_b)
            nc.vector.tensor_mul(out=xt[j], in0=xt[j], in1=rstd_b)

        # ------------- QKV projections -------------
        qT = qkp.tile([128, SEQ], bf16, tag="qT")
        kT = qkp.tile([128, SEQ], bf16, tag="kT")
        for dst, w_sb, b_sb in ((qT, wq_sb, bq_sb), (kT, wk_sb, bk_sb)):
            for sc in range(NSC):
                cs = slice(512 * sc, 512 * (sc + 1))
                ps = mmps.tile([128, 512], f32, tag="mm")
                for j in range(ND):
                    nc.tensor.matmul(
                        out=ps, lhsT=w_sb[:, j, :], rhs=xt[j][:, cs],
                        start=(j == 0), stop=(j == ND - 1),
                    )
                nc.scalar.activation(out=dst[:, cs], in_=ps, func=AF.Identity, bias=b_sb)
        v_aug = []
        for t in range(NTT):
            ps = mmps.tile([128, 2 * DH], f32, tag="mm")
            for j in range(ND):
                nc.tensor.matmul(
                    out=ps, lhsT=xt[j][:, 128 * t : 128 * (t + 1)], rhs=wv_sb[:, j, :],
                    start=(j == 0), stop=(j == ND - 1),
                )
            va = vap.tile([128, 2 * (DH + 1)], bf16, tag="va")
            ones_ap = bass.AP(
                tensor=va.tensor, offset=va.offset + DH,
                ap=[va.ap[0], [DH + 1, 2], [1, 1]],
            )
            nc.vector.memset(ones_ap, 1.0)
            dst_ap = bass.AP(
                tensor=va.tensor, offset=va.offset,
                ap=[va.ap[0], [DH + 1, 2], [1, DH]],
            )
            src_ap = ps.rearrange("p (h e) -> p h e", h=2)
            if has_vb:
                nc.vector.tensor_add(
                    out=dst_ap, in0=src_ap,
                    in1=vb_sb.rearrange("p (h e) -> p h e", h=2),
                )
            else:
                nc.scalar.copy(out=dst_ap, in_=src_ap)
            v_aug.append(va)

        # phase 1 done: release x^T/LN1 pools
        ph1.close()

        # ------------- attention (scores^T, flash-style late normalization) ---
        zT = ztp.tile([128, SEQ], bf16, tag="zT")
        for sc in range(NSC):
            bi, scl = sc // 4, sc % 4
            nt = 4 * (scl + 1)
            tbase = 16 * bi
            scol = 512 * sc
            zA = zps.tile([DH + 1, 512], f32, tag="z")
            zB = zps.tile([DH + 1, 512], f32, tag="z")
            for kt in range(nt):
                t = tbase + kt
                c0 = 128 * (kt - 4 * scl) if kt >= 4 * scl else 0
                sA = scps.tile([128, 512], f32, tag="s")
                sB = scps.tile([128, 512], f32, tag="s")
                nc.tensor.matmul(
                    out=sA[:, c0:], lhsT=kT[0:DH, 128 * t : 128 * (t + 1)],
                    rhs=qT[0:DH, scol + c0 : scol + 512],
                    start=True, stop=True, tile_position=(0, 0),
                )
                nc.tensor.matmul(
                    out=sB[:, c0:], lhsT=kT[DH:128, 128 * t : 128 * (t + 1)],
                    rhs=qT[DH:128, scol + c0 : scol + 512],
                    start=True, stop=True, tile_position=(64, 0),
                )
                pA = pp.tile([128, 512], bf16, tag="pA")
                pB = pp.tile([128, 512], bf16, tag="pB")
                nc.scalar.activation(out=pA[:, c0:], in_=sA[:, c0:], func=AF.Exp, scale=SCALE)
                nc.scalar.activation(out=pB[:, c0:], in_=sB[:, c0:], func=AF.Exp, scale=SCALE)
                if kt >= 4 * scl:  # partially-masked diagonal tile
                    nc.vector.tensor_mul(
                        out=pA[:, c0 : c0 + 128], in0=pA[:, c0 : c0 + 128], in1=tri_sb
                    )
                    nc.vector.tensor_mul(
                        out=pB[:, c0 : c0 + 128], in0=pB[:, c0 : c0 + 128], in1=tri_sb
                    )
                if has_pm:
                    nc.vector.tensor_scalar_mul(
                        out=pA[:, c0:], in0=pA[:, c0:], scalar1=pm_sb[:, t : t + 1]
                    )
                    nc.vector.tensor_scalar_mul(
                        out=pB[:, c0:], in0=pB[:, c0:], scalar1=pm_sb[:, t : t + 1]
                    )
                nc.tensor.matmul(
                    out=zA[:, c0:], lhsT=v_aug[t][:, 0 : DH + 1], rhs=pA[:, c0:],
                    start=(kt == 0), stop=(kt == nt - 1),
                )
                nc.tensor.matmul(
                    out=zB[:, c0:], lhsT=v_aug[t][:, DH + 1 : 2 * (DH + 1)], rhs=pB[:, c0:],
                    start=(kt == 0), stop=(kt == nt - 1),
                )
            # divide by softmax denominator (row DH of each z accumulator)
            for zps_t, half in ((zA, 0), (zB, 1)):
                rip = ripr.tile([1, 512], f32, tag="rip")
                nc.vector.reciprocal(out=rip, in_=zps_t[DH : DH + 1, :])
                bc_ps = auxps.tile([DH, 512], f32, tag="aux")
                nc.tensor.matmul(out=bc_ps, lhsT=ones64_sb, rhs=rip, start=True, stop=True)
                bc_sb = bcr.tile([DH, 512], f32, tag="bcsb")
                nc.scalar.copy(out=bc_sb, in_=bc_ps)
                nc.vector.tensor_mul(
                    out=zT[DH * half : DH * (half + 1), scol : scol + 512],
                    in0=zps_t[0:DH, :], in1=bc_sb,
                )
            nc.sync.dma_start(
                out=a2a_in[128 * sc : 128 * (sc + 1), :],
                in_=zT[:, scol : scol + 512],
            )

        # attention pools done
        mid.close()
        w12 = outer.enter_context(tc.tile_pool(name="w12", bufs=1))
        w1_sb = w12.tile([128, ND, D], bf16)
        w2_sb = w12.tile([128, ND, D], bf16)
        nc.sync.dma_start(out=w1_sb, in_=w1_d.ap().rearrange("(j p) e -> p j e", p=128))
        nc.sync.dma_start(out=w2_sb, in_=w2_d.ap().rearrange("(j p) e -> p j e", p=128))

        # ------------- AllToAll: head-slices -> token-owner cores -------------
        nc.gpsimd.collective_compute(
            "AllToAll",
            ALU.bypass,
            replica_groups=[list(range(NCORES))],
            ins=[a2a_in.opt()],
            outs=[a2a_out.opt()],
        )

        # ------------- residual x + z (token-major, fp32) -------------
        ztok = outer.enter_context(tc.tile_pool(name="ztok", bufs=4))
        xzp = outer.enter_context(tc.tile_pool(name="xzp", bufs=2))
        for r in range(NCORES):
            for i in range(4):
                zt_t = ztok.tile([128, 128], bf16, tag="ztok")
                nc.sync.dma_start_transpose(
                    out=zt_t,
                    in_=a2a_out[128 * r : 128 * (r + 1), 128 * i : 128 * (i + 1)],
                )
                nc.vector.tensor_add(
                    out=xres[i][:, 128 * r : 128 * (r + 1)],
                    in0=xres[i][:, 128 * r : 128 * (r + 1)],
                    in1=zt_t,
                )
        # write (x+z) as bf16 and transpose-load as h2^T
        for i in range(4):
            xz_sb = xzp.tile([128, D], bf16, tag="xz")
            nc.vector.tensor_copy(out=xz_sb, in_=xres[i])
            nc.sync.dma_start(out=xz_dram[128 * i : 128 * (i + 1), :], in_=xz_sb)
        h2p = outer.enter_context(tc.tile_pool(name="h2p", bufs=8))
        h2t = []
        for j in range(ND):
            t = h2p.tile([128, CHUNK], bf16, tag="h2t")
            nc.sync.dma_start_transpose(out=t, in_=xz_dram[:, 128 * j : 128 * (j + 1)])
            h2t.append(t)

        # ------------- LN2 -------------
        rows2 = outer.enter_context(tc.tile_pool(name="rows2", bufs=1))
        bcp2 = outer.enter_context(tc.tile_pool(name="bcp2", bufs=1))
        srow2 = rows2.tile([1, 3 * CHUNK], f32)
        lnbf2 = rows2.tile([1, 2 * CHUNK], bf16)
        sum2_ps = auxps.tile([1, 512], f32, tag="aux")
        sq2_ps = auxps.tile([1, 512], f32, tag="aux")
        for j in range(ND):
            sq_t = sqp.tile([128, 512], bf16, tag="sq")
            nc.vector.tensor_mul(out=sq_t, in0=h2t[j], in1=h2t[j])
            nc.tensor.matmul(out=sum2_ps, lhsT=ones1_sb, rhs=h2t[j],
                             start=(j == 0), stop=(j == ND - 1))
            nc.tensor.matmul(out=sq2_ps, lhsT=ones1_sb, rhs=sq_t,
                             start=(j == 0), stop=(j == ND - 1))
        mu2_r, var2_r, tmp2_r = srow2[:, 0:CHUNK], srow2[:, CHUNK : 2 * CHUNK], srow2[:, 2 * CHUNK : 3 * CHUNK]
        nc.scalar.mul(out=mu2_r, in_=sum2_ps, mul=1.0 / D)
        nc.scalar.mul(out=var2_r, in_=sq2_ps, mul=1.0 / D)
        nc.vector.tensor_mul(out=tmp2_r, in0=mu2_r, in1=mu2_r)
        nc.vector.tensor_sub(out=var2_r, in0=var2_r, in1=tmp2_r)
        nc.scalar.activation(out=tmp2_r, in_=var2_r, func=AF.Ln, bias=eps_sb)
        nc.scalar.activation(out=lnbf2[:, CHUNK : 2 * CHUNK], in_=tmp2_r, func=AF.Exp, scale=-0.5)
        nc.vector.tensor_copy(out=lnbf2[:, 0:CHUNK], in_=mu2_r)
        nc.sync.dma_start(out=ln2rows_dram, in_=lnbf2)
        mu2_b = bcp2.tile([128, CHUNK], bf16, tag="mu2_b")
        rstd2_b = bcp2.tile([128, CHUNK], bf16, tag="rstd2_b")
        nc.gpsimd.dma_start(out=mu2_b, in_=bcast(ln2rows_dram[:, 0:CHUNK], 128))
        nc.gpsimd.dma_start(out=rstd2_b, in_=bcast(ln2rows_dram[:, CHUNK : 2 * CHUNK], 128))
        for j in range(ND):
            nc.vector.tensor_sub(out=h2t[j], in0=h2t[j], in1=mu2_b)
            nc.vector.tensor_mul(out=h2t[j], in0=h2t[j], in1=rstd2_b)

        # ------------- FFN -------------
        atp = outer.enter_context(tc.tile_pool(name="atp", bufs=8))
        ffp = outer.enter_context(tc.tile_pool(name="ffp", bufs=3))
        fftokp = outer.enter_context(tc.tile_pool(name="fftokp", bufs=2))
        outp = outer.enter_context(tc.tile_pool(name="outp", bufs=2))
        aT = []
        for m in range(ND):
            ps = mmps.tile([128, 512], f32, tag="mm")
            for j in range(ND):
                nc.tensor.matmul(
                    out=ps, lhsT=w1_sb[:, j, 128 * m : 128 * (m + 1)], rhs=h2t[j],
                    start=(j == 0), stop=(j == ND - 1),
                )
            a_t = atp.tile([128, CHUNK], bf16, tag="aT")
            nc.scalar.activation(out=a_t, in_=ps, func=AF.Relu, bias=b1_sb[:, m : m + 1])
            aT.append(a_t)
        for dm in range(ND):
            ps = mmps.tile([128, 512], f32, tag="mm")
            for jm in range(ND):
                nc.tensor.matmul(
                    out=ps, lhsT=w2_sb[:, jm, 128 * dm : 128 * (dm + 1)], rhs=aT[jm],
                    start=(jm == 0), stop=(jm == ND - 1),
                )
            ff_sb = ffp.tile([128, CHUNK], bf16, tag="ffT")
            nc.scalar.activation(out=ff_sb, in_=ps, func=AF.Identity, bias=b2_sb[:, dm : dm + 1])
            nc.sync.dma_start(out=fft_dram[128 * dm : 128 * (dm + 1), :], in_=ff_sb)
        # transpose ff back to token-major and add the residual stream
        for i in range(4):
            ff_tok = fftokp.tile([128, D], bf16, tag="fftok")
            nc.sync.dma_start_transpose(out=ff_tok, in_=fft_dram[:, 128 * i : 128 * (i + 1)])
            o_sb = outp.tile([128, D], f32, tag="osb")
            nc.vector.tensor_add(out=o_sb, in0=xres[i], in1=ff_tok)
            nc.sync.dma_start(out=out_d.ap()[128 * i : 128 * (i + 1), :], in_=o_sb)

    nc.compile()
    return nc


_PROGRAM_CACHE = {}


def _get_program(has_pm: bool, has_vb: bool):
    key = (has_pm, has_vb)
    if key not in _PROGRAM_CACHE:
        _PROGRAM_CACHE[key] = _build_program(has_pm, has_vb)
    return _PROGRAM_CACHE[key]


def _run(nc, in_maps, trace=False):
    from concourse import bass_utils

    return bass_utils.run_bass_kernel_spmd(
        nc, in_maps, core_ids=list(range(NCORES)), trace=trace
    )


def prepare_inputs(x, padding_mask, Wq, Wk, Wv, ln1_s, ln1_b, ln2_s, ln2_b, W1, b1, W2, b2):
    x = np.asarray(x, np.float32)
    Wq = np.asarray(Wq, np.float32)
    Wk = np.asarray(Wk, np.float32)
    Wv = np.asarray(Wv, np.float32)
    ln1_s = np.asarray(ln1_s, np.float32)
    ln1_b = np.asarray(ln1_b, np.float32)
    ln2_s = np.asarray(ln2_s, np.float32)
    ln2_b = np.asarray(ln2_b, np.float32)
    W1 = np.asarray(W1, np.float32)
    b1 = np.asarray(b1, np.float32)
    W2 = np.asarray(W2, np.float32)
    b2 = np.asarray(b2, np.float32)
    pm = np.asarray(padding_mask)

    has_pm = not bool(pm.all())
    has_vb = bool(np.any(ln1_b != 0.0))

    x_flat = np.ascontiguousarray(x.reshape(SEQ, D))
    xb = x_flat.astype(BF16)
    w1h = np.ascontiguousarray((ln2_s[:, None] * W1).astype(BF16))
    b1e = (ln2_b @ W1 + b1).astype(np.float32)
    w2h = np.ascontiguousarray(W2.astype(BF16))
    tri = np.triu(np.ones((128, 128), np.float32)).astype(BF16)
    pmf = None
    if has_pm:
        pmf = np.ascontiguousarray(
            np.broadcast_to(pm.astype(np.float32), (B, S)).reshape(SEQ)
        )

    in_maps = []
    for c in range(NCORES):
        h0, h1 = 2 * c, 2 * c + 1
        wcat_q = np.concatenate([Wq[h0], Wq[h1]], axis=1)
        wcat_k = np.concatenate([Wk[h0], Wk[h1]], axis=1)
        wcat_v = np.concatenate([Wv[h0], Wv[h1]], axis=1)
        m = dict(
            xb=xb,
            xres=np.ascontiguousarray(x_flat[CHUNK * c : CHUNK * (c + 1)]),
            wq=np.ascontiguousarray((ln1_s[:, None] * wcat_q).astype(BF16)),
            wk=np.ascontiguousarray((ln1_s[:, None] * wcat_k).astype(BF16)),
            wv=np.ascontiguousarray((ln1_s[:, None] * wcat_v).astype(BF16)),
            bq=(ln1_b @ wcat_q).astype(np.float32),
            bk=(ln1_b @ wcat_k).astype(np.float32),
            w1=w1h,
            b1e=b1e,
            w2=w2h,
            b2e=b2.astype(np.float32),
            trimask=tri,
        )
        if has_pm:
            m["pmf"] = pmf
        if has_vb:
            m["vb"] = (ln1_b @ wcat_v).astype(np.float32)
        in_maps.append(m)
    return in_maps, has_pm, has_vb


def kernel(**inputs):
    in_maps, has_pm, has_vb = prepare_inputs(**inputs)
    nc = _get_program(has_pm, has_vb)
    trace = bool(int(os.environ.get("KERNEL_TRACE", "0")))
    res = _run(nc, in_maps, trace=trace)
    y = np.empty((SEQ, D), np.float32)
    for c in range(NCORES):
        y[CHUNK * c : CHUNK * (c + 1)] = res.results[c]["out"]
    kernel.last_results = res
    return y.reshape(B, S, D)
